# revision 1
# baseline (speedup 1.0000x reference)
"""Multihead attention (B=2, S=2048, E=1024, H=16) on 8 TRN2 cores.

Sharding: tensor-parallel over heads — core c computes heads {2c, 2c+1}
(dout = 128 columns of the QKV projections) for the full sequence, then its
partial contribution to the output projection; the host sums the 8 partials
and adds the output bias.

Device layout (per core):
  activations are pre-transposed on host to x^T [E, B*S] (and rounded to
  bf16 — the bf16 matmuls would round them anyway) so the projection
  matmuls contract E on the partition dim.  QKV projections produce
  Q^T/K^T/V^T [128, 4096] in SBUF (bf16).  Attention per (batch, head)
  computes scores^T [kpos, q] tiles directly (lhsT = K^T slice,
  rhs = Q^T slice), exponentiates on the scalar engine (fp32 psum in,
  bf16 out), and multiplies by V via matmul with lhsT = [V | ones] so the
  softmax denominator falls out of the same accumulation (row 64 of the
  PSUM result).  context^T is normalized with a reciprocal + PE-replicated
  row (kept float32r), and the output projection (float32r = full fp32
  bits) contracts the 128 local head dims.

Emission order interleaves batch-0 attention with batch-1 projections and
batch-1 attention with batch-0 output projection so DMA streaming, PE,
ACT (exp) and DVE stay overlapped across the whole kernel.
"""

import numpy as np
import ml_dtypes

# Problem constants (hardcoded per the task contract).
B, S, E, H = 2, 2048, 1024, 16
D = E // H          # 64
NSEQ = B * S        # 4096
NCORES = 8
DOUT = E // NCORES  # 128 = 2 heads x 64
KE = E // 128       # 8 contraction tiles over E
SEQT = 512          # seq tile for projections / q-block for attention
NST = NSEQ // SEQT  # 8
QB = S // SEQT      # 4 q-blocks per batch
KT = S // 128       # 16 kpos tiles per batch
ISD = float(D) ** -0.5

_PROGRAM = None


# ---------------------------------------------------------------------------
# Workarounds for this walrus build: at most ONE sync wait per instruction is
# reliably accepted ("Too many sync wait commands").  (1) tile's final drain
# gets one wait per logical proc — split them over single-wait SP NOPs;
# (2) a general post-pass moves any instruction's excess waits onto
# preceding same-engine NOPs (engine program order preserves semantics).
# ---------------------------------------------------------------------------


def _install_tile_drain_patch():
    import concourse.mybir as mybir
    import concourse.tile as tile
    from concourse.tile import ScopedClock

    if getattr(tile.TileContext, "_drain_patch_installed", False):
        return

    def _patched_drain_and_barrier(self, tick_clock, wait_clock):
        nc = self.nc
        carrier = nc.sync.nop(nofuse=True)
        wait_clock.add_sem_waits(
            carrier.ins, ScopedClock({None: tick_clock.global_clock})
        )
        si = carrier.ins.sync_info
        waits = list(si.on_wait) if si and si.on_wait else []
        ups = list(si.on_update) if si and si.on_update else []
        if len(waits) > 1:
            carrier.ins.sync_info = mybir.SyncInfo(on_wait=[waits[0]], on_update=ups)
            for w in waits[1:]:
                n2 = nc.sync.nop(nofuse=True)
                n2.ins.sync_info = mybir.SyncInfo(on_wait=[w], on_update=[])
        nc.sync.drain()
        nc.all_engine_barrier()
        popped = nc._tile_sem_poison_stack.pop()
        assert popped is self._sem_poison
        nc.clear_and_free_semaphores(list(self.sems.allocated().values()))
        nc.all_engine_barrier()

    tile.TileContext._drain_and_barrier = _patched_drain_and_barrier
    tile.TileContext._drain_patch_installed = True


MAX_WAITS = 1


def _split_excess_waits(nc):
    import concourse.mybir as mybir

    for bb in nc.main_func.blocks:
        il = list(bb.instructions)
        out = []
        changed = False
        for ins in il:
            si = ins.sync_info
            waits = list(si.on_wait) if si and si.on_wait else []
            if len(waits) > MAX_WAITS:
                changed = True
                extras = waits[: len(waits) - MAX_WAITS]
                keep = waits[len(extras):]
                for i in range(0, len(extras), MAX_WAITS):
                    chunk = extras[i : i + MAX_WAITS]
                    nop = mybir.InstNoOp(
                        name=nc.get_next_instruction_name(), ins=[], outs=[]
                    )
                    nop.engine = ins.engine
                    nop.sync_info = mybir.SyncInfo(on_wait=chunk, on_update=[])
                    out.append(nop)
                ins.sync_info = mybir.SyncInfo(
                    on_wait=keep, on_update=list(si.on_update) if si.on_update else []
                )
            out.append(ins)
        if changed:
            bb.instructions = out


def _build_program():
    import concourse.bass as bass
    import concourse.mybir as mybir
    import concourse.tile as tile
    from concourse.masks import make_identity

    _install_tile_drain_patch()

    f32 = mybir.dt.float32
    f32r = mybir.dt.float32r
    bf16 = mybir.dt.bfloat16

    nc = bass.Bass("TRN2", target_bir_lowering=False, debug=False)

    # DRAM I/O (per core).  Activations/projection weights are bf16.
    xq = nc.dram_tensor("xq", [KE, 128, NSEQ], bf16, kind="ExternalInput").ap()
    xk = nc.dram_tensor("xk", [KE, 128, NSEQ], bf16, kind="ExternalInput").ap()
    xv = nc.dram_tensor("xv", [KE, 128, NSEQ], bf16, kind="ExternalInput").ap()
    wq = nc.dram_tensor("wq", [KE, 128, DOUT], bf16, kind="ExternalInput").ap()
    wk = nc.dram_tensor("wk", [KE, 128, DOUT], bf16, kind="ExternalInput").ap()
    wv = nc.dram_tensor("wv", [KE, 128, DOUT], bf16, kind="ExternalInput").ap()
    wo = nc.dram_tensor("wo", [DOUT, E], f32r, kind="ExternalInput").ap()
    bq = nc.dram_tensor("bq", [DOUT, 1], f32, kind="ExternalInput").ap()
    bk = nc.dram_tensor("bk", [DOUT, 1], f32, kind="ExternalInput").ap()
    bv = nc.dram_tensor("bv", [DOUT, 1], f32, kind="ExternalInput").ap()
    out = nc.dram_tensor("out", [NSEQ, E], f32, kind="ExternalOutput").ap()

    with tile.TileContext(nc) as tc:
        with (
            nc.allow_low_precision(reason="bf16/f32r attention pipeline"),
            tc.tile_pool(name="consts", bufs=1) as consts,
            tc.tile_pool(name="persist", bufs=1) as persist,
            tc.tile_pool(name="xstream", bufs=12) as xstream,
            tc.tile_pool(name="ptp", bufs=8) as ptp,
            tc.tile_pool(name="outp", bufs=4) as outp,
            tc.tile_pool(name="small", bufs=4) as small,
            tc.tile_pool(name="pp_ps", bufs=2, space="PSUM") as pp_ps,
            tc.tile_pool(name="sc_ps", bufs=4, space="PSUM") as sc_ps,
            tc.tile_pool(name="cx_ps", bufs=2, space="PSUM") as cx_ps,
        ):
            # ---- constants / persistent SBUF state ----
            ident_f32 = consts.tile([128, 128], f32)
            make_identity(nc, ident_f32[:])
            ident = consts.tile([128, 128], bf16)
            nc.vector.tensor_copy(ident[:], ident_f32[:])
            onesf = consts.tile([128, 1], f32)
            nc.vector.memset(onesf[:], 1.0)
            ones64 = consts.tile([1, 64], f32r)
            nc.vector.tensor_copy(ones64[:], onesf[0:1, 0:1].broadcast_to([1, 64]))

            w_sb = {}
            b_sb = {}
            for name, wdram, bdram in (("q", wq, bq), ("k", wk, bk), ("v", wv, bv)):
                wt = persist.tile([128, KE, DOUT], bf16, tag=f"w{name}")
                for k in range(KE):
                    nc.sync.dma_start(wt[:, k, :], wdram[k])
                w_sb[name] = wt
                bt = persist.tile([DOUT, 1], f32, tag=f"b{name}")
                nc.sync.dma_start(bt[:], bdram[:])
                b_sb[name] = bt
            wo_sb = persist.tile([DOUT, E], f32r, tag="wo")
            nc.sync.dma_start(wo_sb[:], wo[:])

            qt_sb = persist.tile([128, NSEQ], bf16, tag="qt")
            kt_sb = persist.tile([128, NSEQ], bf16, tag="kt")
            vt_sb = persist.tile([128, NSEQ], bf16, tag="vt")
            xT_sb = {"q": qt_sb, "k": kt_sb, "v": vt_sb}
            # [V | ones] per (kpos chunk, head): [128, 32, 2, 65] bf16
            v_sb = persist.tile([128, NSEQ // 128, 2, D + 1], bf16, tag="vn")
            nc.vector.tensor_copy(
                v_sb[:, :, :, D], onesf[:, 0:1].broadcast_to([128, NSEQ // 128, 2])
            )
            ctxT_sb = persist.tile([128, NSEQ], f32r, tag="ctxT")

            xdram = {"q": xq, "k": xk, "v": xv}

            def proj_step(st):
                sl = bass.ts(st, SEQT)
                for name in ("q", "k", "v"):
                    ps = pp_ps.tile([128, SEQT], f32, tag="pp", name=f"pp{st}{name}")
                    for k in range(KE):
                        xt = xstream.tile([128, SEQT], bf16, tag="xs", name="xt")
                        nc.sync.dma_start(xt[:], xdram[name][k, :, sl])
                        nc.tensor.matmul(
                            ps[:],
                            lhsT=w_sb[name][:, k, :],
                            rhs=xt[:],
                            start=(k == 0),
                            stop=(k == KE - 1),
                        )
                    nc.vector.tensor_scalar_add(
                        xT_sb[name][:, sl], ps[:], b_sb[name][:, 0:1]
                    )
                # transpose this slice of V^T into [V | ones] chunks
                for ci in range(st * (SEQT // 128), (st + 1) * (SEQT // 128)):
                    tp = pp_ps.tile([128, 128], bf16, tag="pp", name="tp")
                    nc.tensor.transpose(
                        tp[:], vt_sb[:, bass.ts(ci, 128)], ident[:]
                    )
                    for h in range(2):
                        nc.vector.tensor_copy(
                            v_sb[:, ci, h, 0:D], tp[:, bass.ts(h, D)]
                        )

            def attn_step(b, qb):
                qsl = bass.ds(b * S + qb * SEQT, SEQT)
                ctx = [None, None]
                for h in range(2):
                    ctx[h] = cx_ps.tile([D + 1, SEQT], f32, tag="cx", name=f"ctx{h}")
                for t in range(KT):
                    ksl = bass.ds(b * S + t * 128, 128)
                    pt = [None, None]
                    for h in range(2):
                        hsl = bass.ts(h, D)
                        sc = sc_ps.tile([128, SEQT], f32, tag="sc", name=f"sc{h}")
                        nc.tensor.matmul(
                            sc[:],
                            lhsT=kt_sb[hsl, ksl],
                            rhs=qt_sb[hsl, qsl],
                            start=True,
                            stop=True,
                        )
                        pt[h] = ptp.tile([128, SEQT], bf16, tag="pt", name=f"pt{h}")
                        nc.scalar.activation(
                            pt[h][:], sc[:], mybir.ActivationFunctionType.Exp,
                            scale=ISD,
                        )
                    for h in range(2):
                        nc.tensor.matmul(
                            ctx[h][:],
                            lhsT=v_sb[:, b * KT + t, h, :],
                            rhs=pt[h][:],
                            start=(t == 0),
                            stop=(t == KT - 1),
                        )
                for h in range(2):
                    hsl = bass.ts(h, D)
                    rec = small.tile([1, SEQT], f32r, tag="rec", name="rec")
                    nc.vector.reciprocal(rec[:], ctx[h][D : D + 1, :])
                    rrep = pp_ps.tile([D, SEQT], f32, tag="pp", name="rrep")
                    nc.tensor.matmul(
                        rrep[:], lhsT=ones64[:], rhs=rec[:], start=True, stop=True
                    )
                    ctmp = small.tile([D, SEQT], f32, tag="ctmp", name="ctmp")
                    nc.vector.tensor_copy(ctmp[:], ctx[h][0:D, :])
                    nc.vector.tensor_tensor(
                        out=ctxT_sb[hsl, qsl],
                        in0=ctmp[:],
                        in1=rrep[:],
                        op=mybir.AluOpType.mult,
                    )

            def outproj_step(m):
                ob = outp.tile([128, E], f32, tag="ob", name="ob")
                for n in range(E // SEQT):
                    ps = pp_ps.tile([128, SEQT], f32, tag="pp", name="ops")
                    nc.tensor.matmul(
                        ps[:],
                        lhsT=ctxT_sb[:, bass.ts(m, 128)],
                        rhs=wo_sb[:, bass.ts(n, SEQT)],
                        start=True,
                        stop=True,
                    )
                    nc.vector.tensor_copy(ob[:, bass.ts(n, SEQT)], ps[:])
                nc.sync.dma_start(out[bass.ts(m, 128), :], ob[:])

            # ---- emission: overlap batches ----
            for st in range(4):           # batch-0 projections
                proj_step(st)
            for qb in range(QB):          # b0 attention // b1 projections
                attn_step(0, qb)
                proj_step(4 + qb)
            for qb in range(QB):          # b1 attention // b0 out-proj
                attn_step(1, qb)
                for m in range(4 * qb, 4 * qb + 4):
                    outproj_step(m)
            for m in range(16, 32):       # b1 out-proj
                outproj_step(m)

    return nc


def _get_program():
    global _PROGRAM
    if _PROGRAM is None:
        _PROGRAM = _build_program()
    return _PROGRAM


def kernel(query, key, value, Wq, bq, Wk, bk, Wv, bv, Wo, bo):
    from concourse.bass_utils import run_bass_kernel_spmd

    nc = _get_program()
    if not getattr(nc, "_waits_split", False):
        _split_excess_waits(nc)
        nc._waits_split = True

    bf = ml_dtypes.bfloat16
    q2 = np.asarray(query, np.float32).reshape(NSEQ, E)
    k2 = np.asarray(key, np.float32).reshape(NSEQ, E)
    v2 = np.asarray(value, np.float32).reshape(NSEQ, E)
    # x^T [E, NSEQ] -> [KE, 128, NSEQ], rounded to bf16 on host (the bf16
    # matmul rounds its inputs anyway)
    xq = np.ascontiguousarray(q2.T).astype(bf).reshape(KE, 128, NSEQ)
    xk = np.ascontiguousarray(k2.T).astype(bf).reshape(KE, 128, NSEQ)
    xv = np.ascontiguousarray(v2.T).astype(bf).reshape(KE, 128, NSEQ)

    Wq = np.asarray(Wq, np.float32)
    Wk = np.asarray(Wk, np.float32)
    Wv = np.asarray(Wv, np.float32)
    Wo = np.asarray(Wo, np.float32)

    in_maps = []
    for c in range(NCORES):
        rsl = slice(DOUT * c, DOUT * (c + 1))
        in_maps.append(
            {
                "xq": xq, "xk": xk, "xv": xv,
                # lhsT for the projections: (W_c)^T [E, DOUT] -> [KE,128,DOUT]
                "wq": np.ascontiguousarray(Wq[rsl, :].T).astype(bf).reshape(KE, 128, DOUT),
                "wk": np.ascontiguousarray(Wk[rsl, :].T).astype(bf).reshape(KE, 128, DOUT),
                "wv": np.ascontiguousarray(Wv[rsl, :].T).astype(bf).reshape(KE, 128, DOUT),
                # rhs for the out-proj: rows c-range of Wo^T  [DOUT, E]
                "wo": np.ascontiguousarray(Wo[:, rsl].T),
                "bq": np.ascontiguousarray(np.asarray(bq, np.float32)[rsl]).reshape(DOUT, 1),
                "bk": np.ascontiguousarray(np.asarray(bk, np.float32)[rsl]).reshape(DOUT, 1),
                "bv": np.ascontiguousarray(np.asarray(bv, np.float32)[rsl]).reshape(DOUT, 1),
            }
        )

    res = run_bass_kernel_spmd(nc, in_maps, list(range(NCORES)), trace=False)
    acc = np.zeros((NSEQ, E), np.float32)
    for c in range(NCORES):
        acc += res.results[c]["out"]
    acc += np.asarray(bo, np.float32)[None, :]
    return acc.reshape(B, S, E)



# revision 3
# speedup vs baseline: 1.1656x; 1.1656x over previous
"""Multihead attention (B=2, S=2048, E=1024, H=16) on 8 TRN2 cores.

Sharding (hybrid data/tensor parallel): core c handles batch c//4 and heads
4g..4g+3 where g = c%4 — each core projects a 256-column slice of Q/K/V for
its batch, runs attention for its 4 heads, and produces its partial
contribution to the output projection.  The host sums 4 partials per batch
and adds the output bias.  Inputs per core are 12 MB (x^T of one batch,
bf16) instead of 24 MB for pure head-parallel — DMA is halved.

Per-core program:
  activations pre-transposed on host to x^T [E, S] bf16; QKV projections
  contract E on the partition dim producing Q^T/K^T/V^T [128, 2(pair), S]
  (partition = within-head-pair dim, so the two heads of a pair occupy
  partitions 0-63 / 64-127).  Projection bias is added on the scalar
  engine (Identity activation, per-partition bias AP) since ACT is idle
  during projections.  V^T is re-transposed to [kpos, d] chunks with a
  trailing ones column ([V | 1]) so the softmax denominator falls out of
  the PV matmul (row 64 of the ctx PSUM tile).

  Attention per (head-pair, q-block 512): for each of 16 kpos tiles the two
  heads' score matmuls (K=64 contraction, base partitions 0/64 → PE row
  tiles (0,0)/(64,0), concurrent in the array) write the two halves of one
  [128, 1024] PSUM tile spanning 2 banks; ONE scalar-engine Exp covers both
  heads, halving ACT instruction count — ACT is the critical engine.  The
  emission runs scores one kpos tile ahead of the PV matmuls so the PE
  FIFO never head-of-line blocks on ACT.  ctx is normalized with
  reciprocal + a PE-replicated row and written to ctxT (f32r).

  The output projection (f32r, full-rate at N=512) is cut into 16 s-tile
  pieces interleaved into later attention units as PE fillers, with
  PSUM→SBUF copies on the (otherwise idle) GPSIMD engine; projection of
  q-block qb+1 is likewise spread through attention of qb.  PSUM budget:
  sc 2x2 banks + ctx 2 + misc 2 = 8 exactly (rrep rides the sc pool).
"""

import numpy as np
import ml_dtypes

# Problem constants (hardcoded per the task contract).
B, S, E, H = 2, 2048, 1024, 16
D = E // H          # 64
NCORES = 8
GPB = 4             # head-groups (cores) per batch
DOUT = E // GPB     # 256 = 4 heads x 64 per core
KE = E // 128       # 8 contraction tiles over E
SEQT = 512          # seq tile for projections / q-block for attention
QB = S // SEQT      # 4 q-blocks
KT = S // 128       # 16 kpos tiles
ISD = float(D) ** -0.5

_PROGRAM = None


# ---------------------------------------------------------------------------
# Workarounds for this walrus build: at most ONE sync wait per instruction is
# reliably accepted ("Too many sync wait commands").  (1) tile's final drain
# gets one wait per logical proc — split them over single-wait SP NOPs;
# (2) a general post-pass moves any instruction's excess waits onto
# preceding same-engine NOPs (engine program order preserves semantics).
# ---------------------------------------------------------------------------


def _install_tile_drain_patch():
    import concourse.mybir as mybir
    import concourse.tile as tile
    from concourse.tile import ScopedClock

    if getattr(tile.TileContext, "_drain_patch_installed", False):
        return

    def _patched_drain_and_barrier(self, tick_clock, wait_clock):
        nc = self.nc
        carrier = nc.sync.nop(nofuse=True)
        wait_clock.add_sem_waits(
            carrier.ins, ScopedClock({None: tick_clock.global_clock})
        )
        si = carrier.ins.sync_info
        waits = list(si.on_wait) if si and si.on_wait else []
        ups = list(si.on_update) if si and si.on_update else []
        if len(waits) > 1:
            carrier.ins.sync_info = mybir.SyncInfo(on_wait=[waits[0]], on_update=ups)
            for w in waits[1:]:
                n2 = nc.sync.nop(nofuse=True)
                n2.ins.sync_info = mybir.SyncInfo(on_wait=[w], on_update=[])
        nc.sync.drain()
        nc.all_engine_barrier()
        popped = nc._tile_sem_poison_stack.pop()
        assert popped is self._sem_poison
        nc.clear_and_free_semaphores(list(self.sems.allocated().values()))
        nc.all_engine_barrier()

    tile.TileContext._drain_and_barrier = _patched_drain_and_barrier
    tile.TileContext._drain_patch_installed = True


MAX_WAITS = 1


def _split_excess_waits(nc):
    import concourse.mybir as mybir

    for bb in nc.main_func.blocks:
        il = list(bb.instructions)
        out = []
        changed = False
        for ins in il:
            si = ins.sync_info
            waits = list(si.on_wait) if si and si.on_wait else []
            if len(waits) > MAX_WAITS:
                changed = True
                extras = waits[: len(waits) - MAX_WAITS]
                keep = waits[len(extras):]
                for i in range(0, len(extras), MAX_WAITS):
                    chunk = extras[i : i + MAX_WAITS]
                    nop = mybir.InstNoOp(
                        name=nc.get_next_instruction_name(), ins=[], outs=[]
                    )
                    nop.engine = ins.engine
                    nop.sync_info = mybir.SyncInfo(on_wait=chunk, on_update=[])
                    out.append(nop)
                ins.sync_info = mybir.SyncInfo(
                    on_wait=keep, on_update=list(si.on_update) if si.on_update else []
                )
            out.append(ins)
        if changed:
            bb.instructions = out
    nc._waits_split = True


def _build_program():
    import concourse.bass as bass
    import concourse.mybir as mybir
    import concourse.tile as tile
    from concourse.masks import make_identity

    _install_tile_drain_patch()

    f32 = mybir.dt.float32
    f32r = mybir.dt.float32r
    bf16 = mybir.dt.bfloat16
    AF = mybir.ActivationFunctionType

    nc = bass.Bass("TRN2", target_bir_lowering=False, debug=False)

    # DRAM I/O (per core).
    xq = nc.dram_tensor("xq", [KE, 128, S], bf16, kind="ExternalInput").ap()
    xk = nc.dram_tensor("xk", [KE, 128, S], bf16, kind="ExternalInput").ap()
    xv = nc.dram_tensor("xv", [KE, 128, S], bf16, kind="ExternalInput").ap()
    wq = nc.dram_tensor("wq", [KE, 128, DOUT], bf16, kind="ExternalInput").ap()
    wk = nc.dram_tensor("wk", [KE, 128, DOUT], bf16, kind="ExternalInput").ap()
    wv = nc.dram_tensor("wv", [KE, 128, DOUT], bf16, kind="ExternalInput").ap()
    wo = nc.dram_tensor("wo", [128, 2, E], f32r, kind="ExternalInput").ap()
    bq = nc.dram_tensor("bq", [128, 2], f32, kind="ExternalInput").ap()
    bk = nc.dram_tensor("bk", [128, 2], f32, kind="ExternalInput").ap()
    bv = nc.dram_tensor("bv", [128, 2], f32, kind="ExternalInput").ap()
    out = nc.dram_tensor("out", [S, E], f32, kind="ExternalOutput").ap()

    with tile.TileContext(nc) as tc:
        with (
            nc.allow_low_precision(reason="bf16/f32r attention pipeline"),
            # PSUM pools in declaration order → bank-aligned offsets:
            # sc 2x2 banks @0, cx 2x1 @8KB, pp 2x1 @12KB = 16KB exactly.
            tc.tile_pool(name="sc_ps", bufs=2, space="PSUM") as sc_ps,
            tc.tile_pool(name="cx_ps", bufs=2, space="PSUM") as cx_ps,
            tc.tile_pool(name="pp_ps", bufs=2, space="PSUM") as pp_ps,
            tc.tile_pool(name="consts", bufs=1) as consts,
            tc.tile_pool(name="persist", bufs=1) as persist,
            tc.tile_pool(name="xstream", bufs=6) as xstream,
            tc.tile_pool(name="ptp", bufs=4) as ptp,
            tc.tile_pool(name="outp", bufs=3) as outp,
            tc.tile_pool(name="small", bufs=4) as small,
        ):
            # ---- constants / persistent SBUF state ----
            ident_f32 = consts.tile([128, 128], f32)
            make_identity(nc, ident_f32[:])
            ident = consts.tile([128, 128], bf16)
            nc.vector.tensor_copy(ident[:], ident_f32[:])
            onesf = consts.tile([128, 1], f32)
            nc.vector.memset(onesf[:], 1.0)
            ones64 = consts.tile([1, D], f32r)
            nc.vector.tensor_copy(ones64[:], onesf[0:1, 0:1].broadcast_to([1, D]))

            w_sb = {}
            b_sb = {}
            for name, wdram, bdram in (("q", wq, bq), ("k", wk, bk), ("v", wv, bv)):
                wt = persist.tile([128, KE, DOUT], bf16, tag=f"w{name}")
                for k in range(KE):
                    nc.sync.dma_start(wt[:, k, :], wdram[k])
                w_sb[name] = wt
                bt = persist.tile([128, 2], f32, tag=f"b{name}")
                nc.sync.dma_start(bt[:], bdram[:])
                b_sb[name] = bt
            wo_sb = persist.tile([128, 2, E], f32r, tag="wo")
            nc.sync.dma_start(wo_sb[:], wo[:])

            qt_sb = persist.tile([128, 2, S], bf16, tag="qt")
            kt_sb = persist.tile([128, 2, S], bf16, tag="kt")
            vt_sb = persist.tile([128, 2, S], bf16, tag="vt")
            xT_sb = {"q": qt_sb, "k": kt_sb, "v": vt_sb}
            # [V | ones] per (kpos chunk, head): [128, 16, 4, 65] bf16
            v_sb = persist.tile([128, KT, 4, D + 1], bf16, tag="vn")
            nc.vector.tensor_copy(
                v_sb[:, :, :, D], onesf[:, 0:1].broadcast_to([128, KT, 4])
            )
            ctxT_sb = persist.tile([128, 2, S], f32r, tag="ctxT")

            xdram = {"q": xq, "k": xk, "v": xv}

            def proj_mms(name, st):
                """Emit one seq-512 projection step: 8 streamed x chunks x
                2 dout halves, accumulating into 2 pp PSUM tiles, then ACT
                bias-adds into the persistent x^T output."""
                sl = bass.ts(st, SEQT)
                ps = [
                    pp_ps.tile([128, SEQT], f32, tag="pp", name=f"ps{name}{st}{j}")
                    for j in range(2)
                ]
                for k in range(KE):
                    xt = xstream.tile([128, SEQT], bf16, tag="xs", name="xt")
                    nc.sync.dma_start(xt[:], xdram[name][k, :, sl])
                    for j in range(2):
                        nc.tensor.matmul(
                            ps[j][:],
                            lhsT=w_sb[name][:, k, bass.ts(j, 128)],
                            rhs=xt[:],
                            start=(k == 0),
                            stop=(k == KE - 1),
                        )
                for j in range(2):
                    nc.scalar.activation(
                        xT_sb[name][:, j, sl], ps[j][:], AF.Identity,
                        bias=b_sb[name][:, j : j + 1],
                    )

            def v_transposes(st):
                """Transpose this seq slice of V^T into [V | ones] chunks
                (PE transpose + ACT copies)."""
                for hp in range(2):
                    for ci in range(4 * st, 4 * st + 4):
                        tp = pp_ps.tile([128, 128], bf16, tag="pp", name="tp")
                        nc.tensor.transpose(
                            tp[:], vt_sb[:, hp, bass.ts(ci, 128)], ident[:]
                        )
                        for e in range(2):
                            nc.scalar.activation(
                                v_sb[:, ci, 2 * hp + e, 0:D],
                                tp[:, bass.ts(e, D)],
                                AF.Copy,
                            )

            def outproj_piece(m):
                """Partial out-proj for s-tile m: 2 e-halves x 2 ctx chunks,
                GPSIMD PSUM→SBUF copies, DMA to DRAM."""
                ob = outp.tile([128, E], f32, tag="ob", name="ob")
                msl = bass.ts(m, 128)
                for et in range(2):
                    ops = pp_ps.tile([128, SEQT], f32, tag="pp", name="ops")
                    for j in range(2):
                        nc.tensor.matmul(
                            ops[:],
                            lhsT=ctxT_sb[:, j, msl],
                            rhs=wo_sb[:, j, bass.ts(et, SEQT)],
                            start=(j == 0),
                            stop=(j == 1),
                        )
                    nc.vector.tensor_copy(ob[:, bass.ts(et, SEQT)], ops[:])
                nc.sync.dma_start(out[msl, :], ob[:])

            def attn_unit(hp, qb, proj_name=None, proj_st=None, pieces=()):
                """One (head-pair, q-block) attention unit.  Scores run one
                kpos tile ahead of PV so PE never blocks on ACT.  Optional
                fillers: a projection step spread through the t-loop
                (occupies both pp bufs), or out-proj pieces at fixed t."""
                qsl = bass.ts(qb, SEQT)
                ctx = [
                    cx_ps.tile([D + 1, SEQT], f32, tag="cx", name=f"ctx{e}")
                    for e in range(2)
                ]
                pieces = list(pieces)

                proj_ps = None
                if proj_name is not None:
                    proj_ps = [
                        pp_ps.tile(
                            [128, SEQT], f32, tag="pp", name=f"ps{proj_name}{proj_st}{j}"
                        )
                        for j in range(2)
                    ]

                def sc_mms(t):
                    sc = sc_ps.tile([128, 2 * SEQT], f32, tag="sc", name=f"sct{t}")
                    ksl = bass.ts(t, 128)
                    for e in range(2):
                        esl = slice(64 * e, 64 * e + 64)
                        nc.tensor.matmul(
                            sc[:, bass.ts(e, SEQT)],
                            lhsT=kt_sb[esl, hp, ksl],
                            rhs=qt_sb[esl, hp, qsl],
                            start=True,
                            stop=True,
                        )
                    return sc

                sc_next = sc_mms(0)
                for t in range(KT):
                    sc_cur = sc_next
                    if t + 1 < KT:
                        sc_next = sc_mms(t + 1)
                    pt = ptp.tile([128, 2 * SEQT], bf16, tag="pt", name="pt")
                    nc.scalar.activation(pt[:], sc_cur[:], AF.Exp, scale=ISD)
                    for e in range(2):
                        nc.tensor.matmul(
                            ctx[e][:],
                            lhsT=v_sb[:, t, 2 * hp + e, :],
                            rhs=pt[:, bass.ts(e, SEQT)],
                            start=(t == 0),
                            stop=(t == KT - 1),
                        )
                    # fillers: spread projection k-chunks / out-proj pieces
                    if proj_name is not None and t % 2 == 0:
                        k = t // 2
                        xt = xstream.tile([128, SEQT], bf16, tag="xs", name="xt")
                        nc.sync.dma_start(
                            xt[:], xdram[proj_name][k, :, bass.ts(proj_st, SEQT)]
                        )
                        for j in range(2):
                            nc.tensor.matmul(
                                proj_ps[j][:],
                                lhsT=w_sb[proj_name][:, k, bass.ts(j, 128)],
                                rhs=xt[:],
                                start=(k == 0),
                                stop=(k == KE - 1),
                            )
                    if pieces and t in (4, 9, 14):
                        outproj_piece(pieces.pop(0))

                if proj_name is not None:
                    for j in range(2):
                        nc.scalar.activation(
                            xT_sb[proj_name][:, j, bass.ts(proj_st, SEQT)],
                            proj_ps[j][:],
                            AF.Identity,
                            bias=b_sb[proj_name][:, j : j + 1],
                        )
                for m in pieces:
                    outproj_piece(m)

                for e in range(2):
                    rec = small.tile([1, SEQT], f32r, tag="rec", name="rec")
                    nc.vector.reciprocal(rec[:], ctx[e][D : D + 1, :])
                    # replicate the reciprocal row across 64 partitions on PE
                    # (rides the sc pool: 2-bank slot, briefly held)
                    rrep = sc_ps.tile([D, SEQT], f32, tag="sc", name="rrep")
                    nc.tensor.matmul(
                        rrep[:], lhsT=ones64[:], rhs=rec[:], start=True, stop=True
                    )
                    ctmp = small.tile([D, SEQT], f32, tag="ctmp", name="ctmp")
                    nc.vector.tensor_copy(ctmp[:], ctx[e][0:D, :])
                    nc.vector.tensor_tensor(
                        out=ctxT_sb[slice(64 * e, 64 * e + 64), hp, qsl],
                        in0=ctmp[:],
                        in1=rrep[:],
                        op=mybir.AluOpType.mult,
                    )

            # ---- emission ----
            # Head: project K and V fully (+ V transposes), then Q for qb0.
            for st in range(QB):
                proj_mms("k", st)
            for st in range(QB):
                proj_mms("v", st)
                v_transposes(st)
            proj_mms("q", 0)

            # Attention with interleaved q-projection and out-proj fillers.
            # ready[qb] = s-tiles whose ctxT completes after q-block qb.
            attn_unit(0, 0, proj_name="q", proj_st=1)
            attn_unit(1, 0)
            attn_unit(0, 1, proj_name="q", proj_st=2)
            attn_unit(1, 1, pieces=(0, 1, 2, 3))
            attn_unit(0, 2, proj_name="q", proj_st=3)
            attn_unit(1, 2, pieces=(4, 5, 6, 7))
            attn_unit(0, 3, pieces=(8, 9))
            attn_unit(1, 3, pieces=(10, 11))
            for m in range(12, 16):
                outproj_piece(m)

    return nc


def _get_program():
    global _PROGRAM
    if _PROGRAM is None:
        _PROGRAM = _build_program()
        if not getattr(_PROGRAM, "_waits_split", False):
            _split_excess_waits(_PROGRAM)
    return _PROGRAM


def kernel(query, key, value, Wq, bq, Wk, bk, Wv, bv, Wo, bo):
    from concourse.bass_utils import run_bass_kernel_spmd

    nc = _get_program()

    bf = ml_dtypes.bfloat16
    q3 = np.asarray(query, np.float32)
    k3 = np.asarray(key, np.float32)
    v3 = np.asarray(value, np.float32)
    # per-batch x^T [E, S] -> [KE, 128, S], rounded to bf16 on host (the
    # bf16 matmul rounds its inputs anyway)
    xs = {}
    for b in range(B):
        xs[b] = {
            "xq": np.ascontiguousarray(q3[b].T).astype(bf).reshape(KE, 128, S),
            "xk": np.ascontiguousarray(k3[b].T).astype(bf).reshape(KE, 128, S),
            "xv": np.ascontiguousarray(v3[b].T).astype(bf).reshape(KE, 128, S),
        }

    Wq = np.asarray(Wq, np.float32)
    Wk = np.asarray(Wk, np.float32)
    Wv = np.asarray(Wv, np.float32)
    Wo = np.asarray(Wo, np.float32)
    bqf = np.asarray(bq, np.float32)
    bkf = np.asarray(bk, np.float32)
    bvf = np.asarray(bv, np.float32)

    wmaps = []
    for g in range(GPB):
        rsl = slice(DOUT * g, DOUT * (g + 1))
        wmaps.append(
            {
                # lhsT for the projections: (W_g)^T [E, DOUT] -> [KE,128,DOUT]
                "wq": np.ascontiguousarray(Wq[rsl, :].T).astype(bf).reshape(KE, 128, DOUT),
                "wk": np.ascontiguousarray(Wk[rsl, :].T).astype(bf).reshape(KE, 128, DOUT),
                "wv": np.ascontiguousarray(Wv[rsl, :].T).astype(bf).reshape(KE, 128, DOUT),
                # out-proj rhs: Wo^T rows rsl as [128, 2, E]
                "wo": np.ascontiguousarray(
                    Wo[:, rsl].T.reshape(2, 128, E).transpose(1, 0, 2)
                ),
                "bq": np.ascontiguousarray(bqf[rsl].reshape(2, 128).T),
                "bk": np.ascontiguousarray(bkf[rsl].reshape(2, 128).T),
                "bv": np.ascontiguousarray(bvf[rsl].reshape(2, 128).T),
            }
        )

    in_maps = []
    for c in range(NCORES):
        b, g = c // GPB, c % GPB
        m = dict(xs[b])
        m.update(wmaps[g])
        in_maps.append(m)

    res = run_bass_kernel_spmd(nc, in_maps, list(range(NCORES)), trace=False)
    bof = np.asarray(bo, np.float32)
    full = np.empty((B, S, E), np.float32)
    for b in range(B):
        acc = res.results[b * GPB]["out"].copy()
        for g in range(1, GPB):
            acc += res.results[b * GPB + g]["out"]
        full[b] = acc + bof[None, :]
    return full


# revision 5
# speedup vs baseline: 1.2509x; 1.0732x over previous
"""Multihead attention (B=2, S=2048, E=1024, H=16) on 8 TRN2 cores.

Sharding (hybrid data/tensor parallel): core c handles batch c//4 and heads
4g..4g+3 where g = c%4 — each core projects a 256-column slice of Q/K/V for
its batch, runs attention for its 4 heads, and produces its partial
contribution to the output projection.  The host sums 4 partials per batch
and adds the output bias.  Inputs per core are 12 MB (x^T of one batch,
bf16) instead of 24 MB for pure head-parallel — DMA is halved.

Per-core program:
  x^T [E, S] bf16 is DMA'd up-front into persistent SBUF (96 seq-major
  chunk DMAs issued at kernel start) so projections are pure PE work and
  never wait on just-in-time transfers.  QKV projections contract E on the
  partition dim producing Q^T/K^T/V^T [128, 2(pair), S] (partition =
  within-head-pair dim).  Projection bias is added on the scalar engine
  (Identity activation, per-partition bias AP) since ACT is idle during
  projections.  V^T is re-transposed to [kpos, d] chunks with a trailing
  ones column ([V | 1]) so the softmax denominator falls out of the PV
  matmul (row 64 of the ctx PSUM tile).

  Attention per (head-pair, q-block 512): for each of 16 kpos tiles the two
  heads' score matmuls (K=64 contraction, base partitions 0/64 → PE row
  tiles (0,0)/(64,0), concurrent in the array) write the two halves of one
  [128, 1024] PSUM tile spanning 2 banks; ONE scalar-engine Exp covers both
  heads, halving ACT instruction count — ACT is the critical engine.  The
  emission runs scores one kpos tile ahead of the PV matmuls so the PE
  FIFO never head-of-line blocks on ACT.  Each unit's softmax
  normalization (reciprocal + PE-replicated row + DVE multiply into ctxT)
  is DEFERRED into the next unit, emitted right after its first score
  matmul, so the serial DVE→PE→DVE chain never stalls the exp stream.

  The output projection (f32r, full-rate at N=512) is cut into 16 s-tile
  pieces interleaved into later attention units as PE fillers; projection
  of q-block qb+1 is likewise spread through attention of qb.  PSUM
  budget: sc 2x2 banks + cx 2 (ctx tiles and deferred-norm rrep share the
  pool) + pp 2 = 8 exactly.
"""

import numpy as np
import ml_dtypes

# Problem constants (hardcoded per the task contract).
B, S, E, H = 2, 2048, 1024, 16
D = E // H          # 64
NCORES = 8
GPB = 4             # head-groups (cores) per batch
DOUT = E // GPB     # 256 = 4 heads x 64 per core
KE = E // 128       # 8 contraction tiles over E
SEQT = 512          # seq tile for projections / q-block for attention
QB = S // SEQT      # 4 q-blocks
KT = S // 128       # 16 kpos tiles
ISD = float(D) ** -0.5

_PROGRAM = None


# ---------------------------------------------------------------------------
# Workarounds for this walrus build: at most ONE sync wait per instruction is
# reliably accepted ("Too many sync wait commands").  (1) tile's final drain
# gets one wait per logical proc — split them over single-wait SP NOPs;
# (2) a general post-pass moves any instruction's excess waits onto
# preceding same-engine NOPs (engine program order preserves semantics).
# ---------------------------------------------------------------------------


def _install_tile_drain_patch():
    import concourse.mybir as mybir
    import concourse.tile as tile
    from concourse.tile import ScopedClock

    if getattr(tile.TileContext, "_drain_patch_installed", False):
        return

    def _patched_drain_and_barrier(self, tick_clock, wait_clock):
        nc = self.nc
        carrier = nc.sync.nop(nofuse=True)
        wait_clock.add_sem_waits(
            carrier.ins, ScopedClock({None: tick_clock.global_clock})
        )
        si = carrier.ins.sync_info
        waits = list(si.on_wait) if si and si.on_wait else []
        ups = list(si.on_update) if si and si.on_update else []
        if len(waits) > 1:
            carrier.ins.sync_info = mybir.SyncInfo(on_wait=[waits[0]], on_update=ups)
            for w in waits[1:]:
                n2 = nc.sync.nop(nofuse=True)
                n2.ins.sync_info = mybir.SyncInfo(on_wait=[w], on_update=[])
        nc.sync.drain()
        nc.all_engine_barrier()
        popped = nc._tile_sem_poison_stack.pop()
        assert popped is self._sem_poison
        nc.clear_and_free_semaphores(list(self.sems.allocated().values()))
        nc.all_engine_barrier()

    tile.TileContext._drain_and_barrier = _patched_drain_and_barrier
    tile.TileContext._drain_patch_installed = True


MAX_WAITS = 1


def _split_excess_waits(nc):
    import concourse.mybir as mybir

    for bb in nc.main_func.blocks:
        il = list(bb.instructions)
        out = []
        changed = False
        for ins in il:
            si = ins.sync_info
            waits = list(si.on_wait) if si and si.on_wait else []
            if len(waits) > MAX_WAITS:
                changed = True
                extras = waits[: len(waits) - MAX_WAITS]
                keep = waits[len(extras):]
                for i in range(0, len(extras), MAX_WAITS):
                    chunk = extras[i : i + MAX_WAITS]
                    nop = mybir.InstNoOp(
                        name=nc.get_next_instruction_name(), ins=[], outs=[]
                    )
                    nop.engine = ins.engine
                    nop.sync_info = mybir.SyncInfo(on_wait=chunk, on_update=[])
                    out.append(nop)
                ins.sync_info = mybir.SyncInfo(
                    on_wait=keep, on_update=list(si.on_update) if si.on_update else []
                )
            out.append(ins)
        if changed:
            bb.instructions = out
    nc._waits_split = True


def _build_program():
    import concourse.bass as bass
    import concourse.mybir as mybir
    import concourse.tile as tile
    from concourse.masks import make_identity

    _install_tile_drain_patch()

    f32 = mybir.dt.float32
    f32r = mybir.dt.float32r
    bf16 = mybir.dt.bfloat16
    AF = mybir.ActivationFunctionType

    nc = bass.Bass("TRN2", target_bir_lowering=False, debug=False)

    # DRAM I/O (per core).
    xq = nc.dram_tensor("xq", [KE, 128, S], bf16, kind="ExternalInput").ap()
    xk = nc.dram_tensor("xk", [KE, 128, S], bf16, kind="ExternalInput").ap()
    xv = nc.dram_tensor("xv", [KE, 128, S], bf16, kind="ExternalInput").ap()
    wq = nc.dram_tensor("wq", [KE, 128, DOUT], bf16, kind="ExternalInput").ap()
    wk = nc.dram_tensor("wk", [KE, 128, DOUT], bf16, kind="ExternalInput").ap()
    wv = nc.dram_tensor("wv", [KE, 128, DOUT], bf16, kind="ExternalInput").ap()
    wo = nc.dram_tensor("wo", [128, 2, E], f32r, kind="ExternalInput").ap()
    bq = nc.dram_tensor("bq", [128, 2], f32, kind="ExternalInput").ap()
    bk = nc.dram_tensor("bk", [128, 2], f32, kind="ExternalInput").ap()
    bv = nc.dram_tensor("bv", [128, 2], f32, kind="ExternalInput").ap()
    out = nc.dram_tensor("out", [S, E], f32, kind="ExternalOutput").ap()

    with tile.TileContext(nc) as tc:
        with (
            nc.allow_low_precision(reason="bf16/f32r attention pipeline"),
            # PSUM pools in declaration order → bank-aligned offsets:
            # sc 2x2 banks @0, cx 2x1 @8KB, pp 2x1 @12KB = 16KB exactly.
            tc.tile_pool(name="sc_ps", bufs=2, space="PSUM") as sc_ps,
            tc.tile_pool(name="cx_ps", bufs=2, space="PSUM") as cx_ps,
            tc.tile_pool(name="pp_ps", bufs=2, space="PSUM") as pp_ps,
            tc.tile_pool(name="consts", bufs=1) as consts,
            tc.tile_pool(name="persist", bufs=1) as persist,
            tc.tile_pool(name="ptp", bufs=3) as ptp,
            tc.tile_pool(name="outp", bufs=2) as outp,
            tc.tile_pool(name="small", bufs=2) as small,
        ):
            # ---- constants / persistent SBUF state ----
            ident_f32 = consts.tile([128, 128], f32)
            make_identity(nc, ident_f32[:])
            ident = consts.tile([128, 128], bf16)
            nc.vector.tensor_copy(ident[:], ident_f32[:])
            onesf = consts.tile([128, 1], f32)
            nc.vector.memset(onesf[:], 1.0)
            ones64 = consts.tile([1, D], f32r)
            nc.vector.tensor_copy(ones64[:], onesf[0:1, 0:1].broadcast_to([1, D]))

            w_sb = {}
            b_sb = {}
            for name, wdram, bdram in (("q", wq, bq), ("k", wk, bk), ("v", wv, bv)):
                wt = persist.tile([128, KE, DOUT], bf16, tag=f"w{name}")
                for k in range(KE):
                    nc.sync.dma_start(wt[:, k, :], wdram[k])
                w_sb[name] = wt
                bt = persist.tile([128, 2], f32, tag=f"b{name}")
                nc.sync.dma_start(bt[:], bdram[:])
                b_sb[name] = bt
            wo_sb = persist.tile([128, 2, E], f32r, tag="wo")
            nc.sync.dma_start(wo_sb[:], wo[:])

            # x^T preloaded whole into SBUF, seq-major chunk DMAs so the
            # earliest projection steps unblock first.
            xdram = {"q": xq, "k": xk, "v": xv}
            x_sb = {}
            for name in ("k", "v", "q"):
                x_sb[name] = persist.tile(
                    [128, KE, S], bf16, tag=f"x{name}", name=f"x{name}_sb"
                )
            for name in ("k", "v", "q"):
                for st in range(QB):
                    sl = bass.ts(st, SEQT)
                    for k in range(KE):
                        nc.sync.dma_start(
                            x_sb[name][:, k, sl], xdram[name][k, :, sl]
                        )

            qt_sb = persist.tile([128, 2, S], bf16, tag="qt")
            kt_sb = persist.tile([128, 2, S], bf16, tag="kt")
            vt_sb = persist.tile([128, 2, S], bf16, tag="vt")
            xT_sb = {"q": qt_sb, "k": kt_sb, "v": vt_sb}
            # [V | ones] per (kpos chunk, head): [128, 16, 4, 65] bf16
            v_sb = persist.tile([128, KT, 4, D + 1], bf16, tag="vn")
            nc.vector.tensor_copy(
                v_sb[:, :, :, D], onesf[:, 0:1].broadcast_to([128, KT, 4])
            )
            ctxT_sb = persist.tile([128, 2, S], f32r, tag="ctxT")

            def proj_mms(name, st):
                """One seq-512 projection step: 8 x chunks x 2 dout halves
                accumulating into 2 pp PSUM tiles, then ACT bias-adds."""
                sl = bass.ts(st, SEQT)
                ps = [
                    pp_ps.tile([128, SEQT], f32, tag="pp", name=f"ps{name}{st}{j}")
                    for j in range(2)
                ]
                for k in range(KE):
                    for j in range(2):
                        nc.tensor.matmul(
                            ps[j][:],
                            lhsT=w_sb[name][:, k, bass.ts(j, 128)],
                            rhs=x_sb[name][:, k, sl],
                            start=(k == 0),
                            stop=(k == KE - 1),
                        )
                for j in range(2):
                    nc.scalar.activation(
                        xT_sb[name][:, j, sl], ps[j][:], AF.Identity,
                        bias=b_sb[name][:, j : j + 1],
                    )

            def v_transposes(st):
                """Transpose this seq slice of V^T into [V | ones] chunks
                (PE transpose + ACT copies)."""
                for hp in range(2):
                    for ci in range(4 * st, 4 * st + 4):
                        tp = pp_ps.tile([128, 128], bf16, tag="pp", name="tp")
                        nc.tensor.transpose(
                            tp[:], vt_sb[:, hp, bass.ts(ci, 128)], ident[:]
                        )
                        for e in range(2):
                            nc.scalar.activation(
                                v_sb[:, ci, 2 * hp + e, 0:D],
                                tp[:, bass.ts(e, D)],
                                AF.Copy,
                            )

            def outproj_piece(m):
                """Partial out-proj for s-tile m: 2 e-halves x 2 ctx chunks,
                DVE PSUM→SBUF copies, DMA to DRAM."""
                ob = outp.tile([128, E], f32, tag="ob", name="ob")
                msl = bass.ts(m, 128)
                for et in range(2):
                    ops = pp_ps.tile([128, SEQT], f32, tag="pp", name="ops")
                    for j in range(2):
                        nc.tensor.matmul(
                            ops[:],
                            lhsT=ctxT_sb[:, j, msl],
                            rhs=wo_sb[:, j, bass.ts(et, SEQT)],
                            start=(j == 0),
                            stop=(j == 1),
                        )
                    nc.vector.tensor_copy(ob[:, bass.ts(et, SEQT)], ops[:])
                nc.sync.dma_start(out[msl, :], ob[:])

            def emit_norm(hp, qb, ctx):
                """Softmax normalization of a finished unit's ctx tiles into
                ctxT.  rrep rides the cx pool (the finished ctx tiles' slots
                free up exactly as the rreps allocate)."""
                qsl = bass.ts(qb, SEQT)
                rec, ctmp = [], []
                for e in range(2):
                    r = small.tile([1, SEQT], f32r, tag="rec", name="rec")
                    nc.vector.reciprocal(r[:], ctx[e][D : D + 1, :])
                    rec.append(r)
                for e in range(2):
                    c = small.tile([D, SEQT], f32, tag="ctmp", name="ctmp")
                    nc.vector.tensor_copy(c[:], ctx[e][0:D, :])
                    ctmp.append(c)
                rrep = []
                for e in range(2):
                    rr = cx_ps.tile([D, SEQT], f32, tag="cx", name="rrep")
                    nc.tensor.matmul(
                        rr[:], lhsT=ones64[:], rhs=rec[e][:], start=True, stop=True
                    )
                    rrep.append(rr)
                for e in range(2):
                    nc.vector.tensor_tensor(
                        out=ctxT_sb[slice(64 * e, 64 * e + 64), hp, qsl],
                        in0=ctmp[e][:],
                        in1=rrep[e][:],
                        op=mybir.AluOpType.mult,
                    )

            def attn_unit(hp, qb, prev=None, proj_name=None, proj_st=None,
                          pieces=()):
                """One (head-pair, q-block) attention unit.  Scores run one
                kpos tile ahead of PV so PE never blocks on ACT.  prev =
                (hp', qb', ctx') of the previous unit — its normalization is
                emitted after this unit's first score matmul.  Optional
                fillers: a projection step spread through the t-loop
                (occupies both pp bufs), or out-proj pieces at fixed t."""
                qsl = bass.ts(qb, SEQT)
                pieces = list(pieces)

                def sc_mms(t):
                    sc = sc_ps.tile([128, 2 * SEQT], f32, tag="sc", name=f"sct{t}")
                    ksl = bass.ts(t, 128)
                    for e in range(2):
                        esl = slice(64 * e, 64 * e + 64)
                        nc.tensor.matmul(
                            sc[:, bass.ts(e, SEQT)],
                            lhsT=kt_sb[esl, hp, ksl],
                            rhs=qt_sb[esl, hp, qsl],
                            start=True,
                            stop=True,
                        )
                    return sc

                sc_next = sc_mms(0)
                if prev is not None:
                    emit_norm(*prev)

                ctx = [
                    cx_ps.tile([D + 1, SEQT], f32, tag="cx", name=f"ctx{e}")
                    for e in range(2)
                ]
                proj_ps = None
                if proj_name is not None:
                    proj_ps = [
                        pp_ps.tile(
                            [128, SEQT], f32, tag="pp",
                            name=f"ps{proj_name}{proj_st}{j}",
                        )
                        for j in range(2)
                    ]

                for t in range(KT):
                    sc_cur = sc_next
                    if t + 1 < KT:
                        sc_next = sc_mms(t + 1)
                    pt = ptp.tile([128, 2 * SEQT], bf16, tag="pt", name="pt")
                    nc.scalar.activation(pt[:], sc_cur[:], AF.Exp, scale=ISD)
                    for e in range(2):
                        nc.tensor.matmul(
                            ctx[e][:],
                            lhsT=v_sb[:, t, 2 * hp + e, :],
                            rhs=pt[:, bass.ts(e, SEQT)],
                            start=(t == 0),
                            stop=(t == KT - 1),
                        )
                    # fillers: spread projection k-chunks / out-proj pieces
                    if proj_name is not None and t % 2 == 0:
                        k = t // 2
                        for j in range(2):
                            nc.tensor.matmul(
                                proj_ps[j][:],
                                lhsT=w_sb[proj_name][:, k, bass.ts(j, 128)],
                                rhs=x_sb[proj_name][:, k, bass.ts(proj_st, SEQT)],
                                start=(k == 0),
                                stop=(k == KE - 1),
                            )
                    if pieces and t in (4, 9, 14):
                        outproj_piece(pieces.pop(0))

                if proj_name is not None:
                    for j in range(2):
                        nc.scalar.activation(
                            xT_sb[proj_name][:, j, bass.ts(proj_st, SEQT)],
                            proj_ps[j][:],
                            AF.Identity,
                            bias=b_sb[proj_name][:, j : j + 1],
                        )
                for m in pieces:
                    outproj_piece(m)
                return (hp, qb, ctx)

            # ---- emission ----
            # Head: project K and V fully (+ V transposes), then Q for qb0.
            for st in range(QB):
                proj_mms("k", st)
            for st in range(QB):
                proj_mms("v", st)
                v_transposes(st)
            proj_mms("q", 0)

            # Attention with deferred normalization and interleaved
            # q-projection / out-proj fillers.
            u = attn_unit(0, 0, proj_name="q", proj_st=1)
            u = attn_unit(1, 0, prev=u)
            u = attn_unit(0, 1, prev=u, proj_name="q", proj_st=2)
            u = attn_unit(1, 1, prev=u, pieces=(0, 1, 2, 3))
            u = attn_unit(0, 2, prev=u, proj_name="q", proj_st=3)
            u = attn_unit(1, 2, prev=u, pieces=(4, 5, 6, 7))
            u = attn_unit(0, 3, prev=u, pieces=(8, 9))
            u = attn_unit(1, 3, prev=u, pieces=(10, 11))
            emit_norm(*u)
            for m in range(12, 16):
                outproj_piece(m)

    return nc


def _get_program():
    global _PROGRAM
    if _PROGRAM is None:
        _PROGRAM = _build_program()
        if not getattr(_PROGRAM, "_waits_split", False):
            _split_excess_waits(_PROGRAM)
    return _PROGRAM


def kernel(query, key, value, Wq, bq, Wk, bk, Wv, bv, Wo, bo):
    from concourse.bass_utils import run_bass_kernel_spmd

    nc = _get_program()

    bf = ml_dtypes.bfloat16
    q3 = np.asarray(query, np.float32)
    k3 = np.asarray(key, np.float32)
    v3 = np.asarray(value, np.float32)
    # per-batch x^T [E, S] -> [KE, 128, S], rounded to bf16 on host (the
    # bf16 matmul rounds its inputs anyway)
    xs = {}
    for b in range(B):
        xs[b] = {
            "xq": np.ascontiguousarray(q3[b].T).astype(bf).reshape(KE, 128, S),
            "xk": np.ascontiguousarray(k3[b].T).astype(bf).reshape(KE, 128, S),
            "xv": np.ascontiguousarray(v3[b].T).astype(bf).reshape(KE, 128, S),
        }

    Wq = np.asarray(Wq, np.float32)
    Wk = np.asarray(Wk, np.float32)
    Wv = np.asarray(Wv, np.float32)
    Wo = np.asarray(Wo, np.float32)
    bqf = np.asarray(bq, np.float32)
    bkf = np.asarray(bk, np.float32)
    bvf = np.asarray(bv, np.float32)

    wmaps = []
    for g in range(GPB):
        rsl = slice(DOUT * g, DOUT * (g + 1))
        wmaps.append(
            {
                # lhsT for the projections: (W_g)^T [E, DOUT] -> [KE,128,DOUT]
                "wq": np.ascontiguousarray(Wq[rsl, :].T).astype(bf).reshape(KE, 128, DOUT),
                "wk": np.ascontiguousarray(Wk[rsl, :].T).astype(bf).reshape(KE, 128, DOUT),
                "wv": np.ascontiguousarray(Wv[rsl, :].T).astype(bf).reshape(KE, 128, DOUT),
                # out-proj rhs: Wo^T rows rsl as [128, 2, E]
                "wo": np.ascontiguousarray(
                    Wo[:, rsl].T.reshape(2, 128, E).transpose(1, 0, 2)
                ),
                "bq": np.ascontiguousarray(bqf[rsl].reshape(2, 128).T),
                "bk": np.ascontiguousarray(bkf[rsl].reshape(2, 128).T),
                "bv": np.ascontiguousarray(bvf[rsl].reshape(2, 128).T),
            }
        )

    in_maps = []
    for c in range(NCORES):
        b, g = c // GPB, c % GPB
        m = dict(xs[b])
        m.update(wmaps[g])
        in_maps.append(m)

    res = run_bass_kernel_spmd(nc, in_maps, list(range(NCORES)), trace=False)
    bof = np.asarray(bo, np.float32)
    full = np.empty((B, S, E), np.float32)
    for b in range(B):
        acc = res.results[b * GPB]["out"].copy()
        for g in range(1, GPB):
            acc += res.results[b * GPB + g]["out"]
        full[b] = acc + bof[None, :]
    return full
